# revision 21
# baseline (speedup 1.0000x reference)
"""CLUB loss kernel for Trainium2, 8 NeuronCores.

Math (reference semantics):
  mu     = head_mu(x)            # BN -> Linear(512,1024) -> ReLU -> BN -> Linear(1024,128)
  logvar = tanh(head_lv(x))
  positive[i,d] = -(mu-y)^2 * 0.5 * exp(-2 lv)
  pair_mse[i,d] = mean_j (y[j,d]-mu[i,d])^2
                = (mu[i,d]-Ey[d])^2 + VarY[d]          (exact algebraic identity)
  negative      = -pair_mse * 0.5 * exp(-lv)
  loss = mean_i( sum_d positive - sum_d negative )

Sharding: model-parallel over the hidden dim (1024 = 8 x 128).  Every core
reads full x (transposed) and computes BN1 stats redundantly (cheaper than a
stats collective), its 128-hidden slice of h/BN2/hn, and rank-1 partial
products for the second matmuls of both heads.  Partials for both heads,
laid out batch-major [1024, 256], go through one ReduceScatter(add), after
which core c owns batch rows [128c, 128c+128) of (mu_pre, lv_pre).  The loss
tail is computed locally per batch shard; per-core partial sums are summed on
the host (the unshard step).
"""

import numpy as np
from contextlib import ExitStack

import concourse.bass as bass
import concourse.bacc as bacc
import concourse.tile as tile
import concourse.mybir as mybir
from concourse.bass_utils import run_bass_kernel_spmd

N, XD, YD, HID = 1024, 512, 128, 1024
NCORES = 8
HS = HID // NCORES     # hidden slice per core
BS = N // NCORES       # batch rows per core after scatter
EPS = 1e-5
F32 = mybir.dt.float32


def _program(ctx, tc, io, out_ap, stage=99, dbg_ap=None):
    # stage: debug truncation point; 99 = full kernel. dbg_ap: [128, 2048]
    # DRAM debug output for truncated runs.
    nc = tc.nc
    A = mybir.AluOpType
    AF = mybir.ActivationFunctionType
    XT, YN, YS, W1, W2, G1B1, V2D, C2R = (
        io[k] for k in ["xT", "yN", "ys", "w1", "w2", "g1b1", "vec2", "c2row"]
    )

    sb = ctx.enter_context(tc.tile_pool(name="sb", bufs=1))
    ps1 = ctx.enter_context(tc.tile_pool(name="ps1", bufs=2, space="PSUM"))
    ps2 = ctx.enter_context(tc.tile_pool(name="ps2", bufs=4, space="PSUM"))
    psm = ctx.enter_context(tc.tile_pool(name="psm", bufs=1, space="PSUM"))
    dram = ctx.enter_context(tc.tile_pool(name="dram", bufs=1, space="DRAM"))

    # ---- constants / params --------------------------------------------
    ones_col = sb.tile([128, 1], F32, tag="ones_col")
    nc.vector.memset(ones_col[:], 1.0)
    ones_row = sb.tile([1, 128], F32, tag="ones_row")
    nc.vector.memset(ones_row[:], 1.0)

    P1 = sb.tile([128, 16], F32, tag="p1")       # (g1mu,b1mu,g1lv,b1lv) x 4 chunks
    for k in range(4):
        nc.sync.dma_start(P1[:, 4 * k:4 * k + 4], G1B1[128 * k:128 * (k + 1), :])
    V2 = sb.tile([128, 6], F32, tag="v2")        # c1mu,c1lv,g2mu,b2mu,g2lv,b2lv
    nc.sync.dma_start(V2[:], V2D[:, :])
    row512 = sb.tile([1, 512], F32, tag="row512")  # [c2mu | c2lv | Ey | VarY]
    nc.sync.dma_start(row512[:, 0:256], C2R[:, :])

    W1t = []
    for k in range(4):
        t = sb.tile([128, 2 * HS], F32, tag=f"w1_{k}")
        nc.sync.dma_start(t[:], W1[128 * k:128 * (k + 1), :])
        W1t.append(t)
    W2t = sb.tile([128, 2 * YD], F32, tag="w2")
    nc.sync.dma_start(W2t[:], W2[:, :])
    YSt = sb.tile([BS, YD], F32, tag="ys")
    nc.sync.dma_start(YSt[:], YS[:, :])

    # ---- x load + BN1 ---------------------------------------------------
    Xt = []
    for k in range(4):
        t = sb.tile([128, N], F32, tag=f"x{k}")
        nc.sync.dma_start(t[:], XT[128 * k:128 * (k + 1), :])
        Xt.append(t)

    MV1 = sb.tile([128, 8], F32, tag="mv1")      # (mean, var) x 4 chunks
    for k in range(4):
        s6 = sb.tile([128, 12], F32, tag=f"s6_{k}")
        for h in range(2):
            nc.vector.bn_stats(
                s6[:, 6 * h:6 * h + 6], Xt[k][:, 512 * h:512 * (h + 1)]
            )
        nc.vector.bn_aggr(MV1[:, 2 * k:2 * k + 2], s6[:])

    # invstd for all 4 chunks at once: [128, 4]
    vp1 = sb.tile([128, 4], F32, tag="vp1")
    nc.vector.tensor_scalar_add(vp1[:], MV1[:, 1:8:2], EPS)
    rc1 = sb.tile([128, 4], F32, tag="rc1")
    nc.vector.reciprocal(rc1[:], vp1[:])
    inv1 = sb.tile([128, 4], F32, tag="inv1")
    nc.scalar.sqrt(inv1[:], rc1[:])

    # per-head affine  xn = x*Ah + Bh ;  Ah = invstd*g1h, Bh = b1h - mean*Ah
    Amu = sb.tile([128, 4], F32, tag="amu")
    Bmu = sb.tile([128, 4], F32, tag="bmu")
    Alv = sb.tile([128, 4], F32, tag="alv")
    Blv = sb.tile([128, 4], F32, tag="blv")
    tmp1 = sb.tile([128, 4], F32, tag="tmp1")
    tmp2 = sb.tile([128, 4], F32, tag="tmp2")
    nc.vector.tensor_tensor(Amu[:], inv1[:], P1[:, 0:16:4], op=A.mult)
    nc.vector.tensor_tensor(tmp1[:], MV1[:, 0:8:2], Amu[:], op=A.mult)
    nc.vector.tensor_tensor(Bmu[:], P1[:, 1:16:4], tmp1[:], op=A.subtract)
    nc.vector.tensor_tensor(Alv[:], inv1[:], P1[:, 2:16:4], op=A.mult)
    nc.vector.tensor_tensor(tmp2[:], MV1[:, 0:8:2], Alv[:], op=A.mult)
    nc.vector.tensor_tensor(Blv[:], P1[:, 3:16:4], tmp2[:], op=A.subtract)

    XNmu, XNlv = [], []
    for k in range(4):
        t = sb.tile([128, N], F32, tag=f"xnmu{k}")
        nc.vector.tensor_scalar(
            t[:], Xt[k][:], Amu[:, k:k + 1], Bmu[:, k:k + 1],
            op0=A.mult, op1=A.add,
        )
        XNmu.append(t)
        t = sb.tile([128, N], F32, tag=f"xnlv{k}")
        nc.scalar.activation(
            t[:], Xt[k][:], AF.Identity,
            bias=Blv[:, k:k + 1], scale=Alv[:, k:k + 1],
        )
        XNlv.append(t)
    XN = [XNmu, XNlv]

    if stage <= 2:
        nc.sync.dma_start(dbg_ap[:, 0:8], MV1[:])
        nc.sync.dma_start(dbg_ap[:, 8:12], Amu[:])
        nc.sync.dma_start(dbg_ap[:, 12:16], Bmu[:])
        if stage == 2:
            nc.sync.dma_start(dbg_ap[:, 16:1040], XNmu[0][:])
            nc.sync.dma_start(dbg_ap[:, 1040:2048], XNlv[1][:, 0:1008])
        return

    # ---- mm1 + ReLU + BN2 + hn -----------------------------------------
    H = [
        sb.tile([128, N], F32, tag="hmu", name="hmu"),
        sb.tile([128, N], F32, tag="hlv", name="hlv"),
    ]
    MV2 = sb.tile([128, 4], F32, tag="mv2")
    for head in range(2):
        for half in range(2):
            pm = ps1.tile([128, 512], F32, tag="mm1")
            for k in range(4):
                nc.tensor.matmul(
                    pm[:],
                    lhsT=W1t[k][:, head * HS:(head + 1) * HS],
                    rhs=XN[head][k][:, half * 512:(half + 1) * 512],
                    start=(k == 0), stop=(k == 3),
                )
            nc.scalar.activation(
                H[head][:, half * 512:(half + 1) * 512], pm[:], AF.Relu,
                bias=V2[:, head:head + 1], scale=1.0,
            )
        s6h = sb.tile([128, 12], F32, tag=f"s6h{head}")
        for h in range(2):
            nc.vector.bn_stats(
                s6h[:, 6 * h:6 * h + 6], H[head][:, 512 * h:512 * (h + 1)]
            )
        nc.vector.bn_aggr(MV2[:, 2 * head:2 * head + 2], s6h[:])

    vp2 = sb.tile([128, 2], F32, tag="vp2")
    nc.vector.tensor_scalar_add(vp2[:], MV2[:, 1:4:2], EPS)
    rc2 = sb.tile([128, 2], F32, tag="rc2")
    nc.vector.reciprocal(rc2[:], vp2[:])
    inv2 = sb.tile([128, 2], F32, tag="inv2")
    nc.scalar.sqrt(inv2[:], rc2[:])

    A2 = sb.tile([128, 2], F32, tag="a2")
    B2 = sb.tile([128, 2], F32, tag="b2")
    tmp3 = sb.tile([128, 2], F32, tag="tmp3")
    # A2[:,h] = inv2[:,h]*g2h ; B2[:,h] = b2h - mean2h*A2[:,h]
    nc.vector.tensor_tensor(A2[:], inv2[:], V2[:, 2:6:2], op=A.mult)
    nc.vector.tensor_tensor(tmp3[:], MV2[:, 0:4:2], A2[:], op=A.mult)
    nc.vector.tensor_tensor(B2[:], V2[:, 3:6:2], tmp3[:], op=A.subtract)

    HN = [
        sb.tile([128, N], F32, tag="hnmu", name="hnmu"),
        sb.tile([128, N], F32, tag="hnlv", name="hnlv"),
    ]
    nc.vector.tensor_scalar(
        HN[0][:], H[0][:], A2[:, 0:1], B2[:, 0:1], op0=A.mult, op1=A.add
    )
    nc.scalar.activation(
        HN[1][:], H[1][:], AF.Identity, bias=B2[:, 1:2], scale=A2[:, 1:2]
    )

    if stage <= 4:
        nc.sync.dma_start(dbg_ap[:, 0:1024], HN[0][:])
        nc.sync.dma_start(dbg_ap[:, 1024:2048], HN[1][:])
        return

    # ---- mm2: batch-major partials, both heads -------------------------
    # n = head*8 + j  ->  PO[n//4][:, (n%4)*128 : ...]
    PO = [ps2.tile([128, 512], F32, tag="po", name=f"po{t}") for t in range(4)]
    for n in range(16):
        head, j = divmod(n, 8)
        t, q = divmod(n, 4)
        nc.tensor.matmul(
            PO[t][:, q * 128:(q + 1) * 128],
            lhsT=HN[head][:, j * 128:(j + 1) * 128],
            rhs=W2t[:, head * YD:(head + 1) * YD],
            start=True, stop=True,
        )
    OUTS = sb.tile([128, 2048], F32, tag="outs")
    for t in range(4):
        if t % 2 == 0:
            nc.vector.tensor_copy(OUTS[:, t * 512:(t + 1) * 512], PO[t][:])
        else:
            nc.scalar.copy(OUTS[:, t * 512:(t + 1) * 512], PO[t][:])

    if stage <= 5:
        nc.sync.dma_start(dbg_ap[:, 0:2048], OUTS[:])
        return

    rs_in = dram.tile([N, 2 * YD], F32, tag="rs_in")
    rs_out = dram.tile([BS, 2 * YD], F32, tag="rs_out")
    for n in range(16):
        head, j = divmod(n, 8)
        nc.sync.dma_start(
            rs_in[j * BS:(j + 1) * BS, head * YD:(head + 1) * YD],
            OUTS[:, n * 128:(n + 1) * 128],
        )
    nc.gpsimd.collective_compute(
        "ReduceScatter",
        A.add,
        replica_groups=[list(range(NCORES))],
        ins=[rs_in[:].opt()],
        outs=[rs_out[:].opt()],
    )

    if stage <= 6:
        R6 = sb.tile([BS, 256], F32, tag="r6")
        nc.sync.dma_start(R6[:], rs_out[:])
        nc.sync.dma_start(dbg_ap[0:BS, 0:256], R6[:])
        return

    # ---- y stats (overlaps everything above) ---------------------------
    PYS = psm.tile([1, 256], F32, tag="aux")
    for j in range(8):
        ysq = sb.tile([128, 256], F32, tag=f"ysq{j}")
        nc.sync.dma_start(ysq[:, 0:128], YN[j * 128:(j + 1) * 128, :])
        nc.scalar.square(ysq[:, 128:256], ysq[:, 0:128])
        nc.tensor.matmul(
            PYS[:], lhsT=ones_col[:], rhs=ysq[:], start=(j == 0), stop=(j == 7)
        )
    # Ey and VarY rows
    nc.scalar.mul(row512[:, 256:384], PYS[:, 0:128], 1.0 / N)
    ey2 = sb.tile([1, 128], F32, tag="ey2")
    nc.scalar.mul(ey2[:], PYS[:, 128:256], 1.0 / N)
    eysq = sb.tile([1, 128], F32, tag="eysq")
    nc.vector.tensor_tensor(eysq[:], row512[:, 256:384], row512[:, 256:384], op=A.mult)
    nc.vector.tensor_tensor(row512[:, 384:512], ey2[:], eysq[:], op=A.subtract)

    # broadcast [c2mu | c2lv | Ey | VarY] along 128 batch partitions
    BC = psm.tile([128, 512], F32, tag="bc")
    nc.tensor.matmul(BC[:], lhsT=ones_row[:], rhs=row512[:], start=True, stop=True)

    if stage <= 7:
        BCs = sb.tile([128, 512], F32, tag="bcs")
        nc.vector.tensor_copy(BCs[:], BC[:])
        nc.sync.dma_start(dbg_ap[:, 0:512], BCs[:])
        return

    # ---- post-scatter loss tail ----------------------------------------
    R = sb.tile([BS, 256], F32, tag="rres")
    nc.sync.dma_start(R[:], rs_out[:])

    mu = sb.tile([BS, YD], F32, tag="mu")
    nc.vector.tensor_tensor(mu[:], R[:, 0:128], BC[:, 0:128], op=A.add)
    plv = sb.tile([BS, YD], F32, tag="plv")
    nc.vector.tensor_tensor(plv[:], R[:, 128:256], BC[:, 128:256], op=A.add)
    lvt = sb.tile([BS, YD], F32, tag="lvt")
    nc.scalar.activation(lvt[:], plv[:], AF.Tanh)
    E1 = sb.tile([BS, YD], F32, tag="e1")
    nc.scalar.activation(E1[:], lvt[:], AF.Exp, scale=-1.0)
    E2 = sb.tile([BS, YD], F32, tag="e2")
    nc.vector.tensor_tensor(E2[:], E1[:], E1[:], op=A.mult)

    dm = sb.tile([BS, YD], F32, tag="dm")
    nc.vector.tensor_tensor(dm[:], mu[:], BC[:, 256:384], op=A.subtract)
    q1 = sb.tile([BS, YD], F32, tag="q1")
    nc.vector.tensor_tensor(q1[:], dm[:], dm[:], op=A.mult)
    q2 = sb.tile([BS, YD], F32, tag="q2")
    nc.vector.tensor_tensor(q2[:], q1[:], BC[:, 384:512], op=A.add)

    if stage <= 8:
        nc.sync.dma_start(dbg_ap[:, 0:128], mu[:])
        nc.sync.dma_start(dbg_ap[:, 128:256], E1[:])
        nc.sync.dma_start(dbg_ap[:, 256:384], E2[:])
        nc.sync.dma_start(dbg_ap[:, 384:512], q2[:])
        return

    scrA = sb.tile([BS, YD], F32, tag="scrA")
    nc.vector.tensor_tensor(scrA[:], E1[:], q2[:], op=A.mult)
    uacc = sb.tile([BS, 1], F32, tag="uacc")
    nc.vector.tensor_reduce(uacc[:], scrA[:], axis=mybir.AxisListType.X, op=A.add)
    dd = sb.tile([BS, YD], F32, tag="dd")
    nc.vector.tensor_tensor(dd[:], mu[:], YSt[:], op=A.subtract)
    wd = sb.tile([BS, YD], F32, tag="wd")
    nc.vector.tensor_tensor(wd[:], E2[:], dd[:], op=A.mult)
    scrB = sb.tile([BS, YD], F32, tag="scrB")
    nc.vector.tensor_tensor(scrB[:], wd[:], dd[:], op=A.mult)
    vacc = sb.tile([BS, 1], F32, tag="vacc")
    nc.vector.tensor_reduce(vacc[:], scrB[:], axis=mybir.AxisListType.X, op=A.add)
    rl = sb.tile([BS, 1], F32, tag="rl")
    nc.vector.tensor_tensor(rl[:], uacc[:], vacc[:], op=A.subtract)

    if stage <= 9:
        nc.sync.dma_start(dbg_ap[:, 0:1], uacc[:])
        nc.sync.dma_start(dbg_ap[:, 1:2], vacc[:])
        nc.sync.dma_start(dbg_ap[:, 2:3], rl[:])
        return

    PF = psm.tile([1, 1], F32, tag="aux")
    nc.tensor.matmul(PF[:], lhsT=rl[:], rhs=ones_col[:], start=True, stop=True)
    res = sb.tile([1, 1], F32, tag="res")
    nc.scalar.mul(res[:], PF[:], 0.5 / N)
    nc.sync.dma_start(out_ap[:, :], res[:])


_NC_CACHE = {}


def build(stage=99):
    if stage in _NC_CACHE:
        return _NC_CACHE[stage]
    nc = bacc.Bacc(
        "TRN2", target_bir_lowering=False, debug=False, num_devices=NCORES
    )
    io = {}

    def inp(name, shape):
        io[name] = nc.dram_tensor(name, list(shape), F32, kind="ExternalInput").ap()

    inp("xT", (XD, N))
    inp("yN", (N, YD))
    inp("ys", (BS, YD))
    inp("w1", (XD, 2 * HS))
    inp("w2", (HS, 2 * YD))
    inp("g1b1", (XD, 4))
    inp("vec2", (HS, 6))
    inp("c2row", (1, 2 * YD))
    out_ap = nc.dram_tensor("out", [1, 1], F32, kind="ExternalOutput").ap()
    dbg_ap = None
    if stage < 99:
        dbg_ap = nc.dram_tensor("dbg", [128, 2048], F32, kind="ExternalOutput").ap()

    with tile.TileContext(nc) as tc, ExitStack() as ctx:
        _program(ctx, tc, io, out_ap, stage=stage, dbg_ap=dbg_ap)
    nc.compile()
    _NC_CACHE[stage] = nc
    return nc


def make_in_maps(
    x_samples, y_samples,
    mu_g1, mu_b1, mu_W1, mu_c1, mu_g2, mu_b2, mu_W2, mu_c2,
    lv_g1, lv_b1, lv_W1, lv_c1, lv_g2, lv_b2, lv_W2, lv_c2,
):
    f = np.float32
    xT = np.ascontiguousarray(np.asarray(x_samples, f).T)          # [512, 1024]
    yN = np.ascontiguousarray(np.asarray(y_samples, f))            # [1024, 128]
    g1b1 = np.ascontiguousarray(
        np.stack([mu_g1, mu_b1, lv_g1, lv_b1], axis=1).astype(f)
    )                                                              # [512, 4]
    c2row = np.ascontiguousarray(
        np.concatenate([mu_c2, lv_c2])[None, :].astype(f)
    )                                                              # [1, 256]
    in_maps = []
    for c in range(NCORES):
        hs = slice(c * HS, (c + 1) * HS)
        bs = slice(c * BS, (c + 1) * BS)
        w1 = np.ascontiguousarray(
            np.concatenate([mu_W1[:, hs], lv_W1[:, hs]], axis=1).astype(f)
        )                                                          # [512, 256]
        w2 = np.ascontiguousarray(
            np.concatenate([mu_W2[hs, :], lv_W2[hs, :]], axis=1).astype(f)
        )                                                          # [128, 256]
        vec2 = np.ascontiguousarray(
            np.stack(
                [mu_c1[hs], lv_c1[hs], mu_g2[hs], mu_b2[hs], lv_g2[hs], lv_b2[hs]],
                axis=1,
            ).astype(f)
        )                                                          # [128, 6]
        ys = np.ascontiguousarray(yN[bs, :])                       # [128, 128]
        in_maps.append(
            dict(xT=xT, yN=yN, ys=ys, w1=w1, w2=w2, g1b1=g1b1, vec2=vec2, c2row=c2row)
        )
    return in_maps


def run_on_hw(in_maps, trace=False, stage=99, **kw):
    nc = build(stage)
    return run_bass_kernel_spmd(nc, in_maps, list(range(NCORES)), trace=trace, **kw)


def kernel(**inputs) -> np.ndarray:
    in_maps = make_in_maps(**inputs)
    res = run_on_hw(in_maps)
    total = np.float32(0.0)
    for r in res.results:
        total += np.float32(r["out"].reshape(-1)[0])
    return np.asarray(total, dtype=np.float32)


# revision 22
# speedup vs baseline: 1.0147x; 1.0147x over previous
"""CLUB loss kernel for Trainium2, 8 NeuronCores.

Math (reference semantics):
  mu     = head_mu(x)            # BN -> Linear(512,1024) -> ReLU -> BN -> Linear(1024,128)
  logvar = tanh(head_lv(x))
  positive[i,d] = -(mu-y)^2 * 0.5 * exp(-2 lv)
  pair_mse[i,d] = mean_j (y[j,d]-mu[i,d])^2
                = (mu[i,d]-Ey[d])^2 + VarY[d]          (exact algebraic identity)
  negative      = -pair_mse * 0.5 * exp(-lv)
  loss = mean_i( sum_d positive - sum_d negative )

Sharding: model-parallel over the hidden dim (1024 = 8 x 128).  Every core
reads full x (transposed) and computes BN1 stats redundantly (cheaper than a
stats collective), its 128-hidden slice of h/BN2/hn, and rank-1 partial
products for the second matmuls of both heads.  Partials for both heads,
laid out batch-major [1024, 256], go through one AllToAll; each core then
sums the 8 received partial slabs for its own batch shard and computes the
loss tail locally.  Per-core partial sums are summed on the host (unshard).

Matmuls run in bf16 (fp32 matmul is quarter-rate on PE); batch-norm
statistics and all loss arithmetic stay fp32.
"""

import numpy as np
from contextlib import ExitStack

import concourse.bass as bass
import concourse.bacc as bacc
import concourse.tile as tile
import concourse.mybir as mybir
from concourse.bass_utils import run_bass_kernel_spmd

N, XD, YD, HID = 1024, 512, 128, 1024
NCORES = 8
HS = HID // NCORES     # hidden slice per core
BS = N // NCORES       # batch rows per core after all-to-all
EPS = 1e-5
F32 = mybir.dt.float32
BF16 = mybir.dt.bfloat16


def _program(ctx, tc, io, out_ap):
    nc = tc.nc
    A = mybir.AluOpType
    AF = mybir.ActivationFunctionType
    XT, YN, YS, W1, W2, G1B1, V2D, C2R = (
        io[k] for k in ["xT", "yN", "ys", "w1", "w2", "g1b1", "vec2", "c2row"]
    )

    sb = ctx.enter_context(tc.tile_pool(name="sb", bufs=1))
    ps1 = ctx.enter_context(tc.tile_pool(name="ps1", bufs=2, space="PSUM"))
    ps2 = ctx.enter_context(tc.tile_pool(name="ps2", bufs=4, space="PSUM"))
    psm = ctx.enter_context(tc.tile_pool(name="psm", bufs=1, space="PSUM"))
    dram = ctx.enter_context(tc.tile_pool(name="dram", bufs=1, space="DRAM"))

    # ---- x load first (critical path), split across both HWDGE rings ---
    Xt = []
    for k in range(4):
        t = sb.tile([128, N], F32, tag=f"x{k}", name=f"x{k}")
        eng = nc.sync if k < 2 else nc.scalar
        eng.dma_start(t[:], XT[128 * k:128 * (k + 1), :])
        Xt.append(t)

    # ---- params ---------------------------------------------------------
    ones_col = sb.tile([128, 1], F32, tag="ones_col")
    nc.vector.memset(ones_col[:], 1.0)
    ones_row = sb.tile([1, 128], F32, tag="ones_row")
    nc.vector.memset(ones_row[:], 1.0)

    P1 = sb.tile([128, 16], F32, tag="p1")       # (g1mu,b1mu,g1lv,b1lv) x 4 chunks
    for k in range(4):
        nc.sync.dma_start(P1[:, 4 * k:4 * k + 4], G1B1[128 * k:128 * (k + 1), :])
    V2 = sb.tile([128, 6], F32, tag="v2")        # c1mu,c1lv,g2mu,b2mu,g2lv,b2lv
    nc.sync.dma_start(V2[:], V2D[:, :])
    row512 = sb.tile([1, 512], F32, tag="row512")  # [c2mu | c2lv | Ey | VarY]
    nc.sync.dma_start(row512[:, 0:256], C2R[:, :])

    W1b = []
    for k in range(4):
        t32 = sb.tile([128, 2 * HS], F32, tag="w1f32", name=f"w1f{k}")
        nc.scalar.dma_start(t32[:], W1[128 * k:128 * (k + 1), :])
        tb = sb.tile([128, 2 * HS], BF16, tag=f"w1b{k}", name=f"w1b{k}")
        eng = nc.vector if k % 2 == 0 else nc.gpsimd
        eng.tensor_copy(tb[:], t32[:])
        W1b.append(tb)
    W2f = sb.tile([128, 2 * YD], F32, tag="w2f")
    nc.scalar.dma_start(W2f[:], W2[:, :])
    W2b = sb.tile([128, 2 * YD], BF16, tag="w2b")
    nc.gpsimd.tensor_copy(W2b[:], W2f[:])
    YSt = sb.tile([BS, YD], F32, tag="ys")
    nc.sync.dma_start(YSt[:], YS[:, :])

    # ---- BN1 ------------------------------------------------------------
    MV1 = sb.tile([128, 8], F32, tag="mv1")      # (mean, var) x 4 chunks
    for k in range(4):
        s6 = sb.tile([128, 12], F32, tag=f"s6_{k}", name=f"s6_{k}")
        for h in range(2):
            nc.vector.bn_stats(
                s6[:, 6 * h:6 * h + 6], Xt[k][:, 512 * h:512 * (h + 1)]
            )
        nc.vector.bn_aggr(MV1[:, 2 * k:2 * k + 2], s6[:])

    # invstd for all 4 chunks at once: [128, 4]
    vp1 = sb.tile([128, 4], F32, tag="vp1")
    nc.vector.tensor_scalar_add(vp1[:], MV1[:, 1:8:2], EPS)
    rc1 = sb.tile([128, 4], F32, tag="rc1")
    nc.vector.reciprocal(rc1[:], vp1[:])
    inv1 = sb.tile([128, 4], F32, tag="inv1")
    nc.scalar.sqrt(inv1[:], rc1[:])

    # per-head affine  xn = x*Ah + Bh ;  Ah = invstd*g1h, Bh = b1h - mean*Ah
    Amu = sb.tile([128, 4], F32, tag="amu")
    Bmu = sb.tile([128, 4], F32, tag="bmu")
    Alv = sb.tile([128, 4], F32, tag="alv")
    Blv = sb.tile([128, 4], F32, tag="blv")
    tmp1 = sb.tile([128, 4], F32, tag="tmp1")
    tmp2 = sb.tile([128, 4], F32, tag="tmp2")
    nc.vector.tensor_tensor(Amu[:], inv1[:], P1[:, 0:16:4], op=A.mult)
    nc.vector.tensor_tensor(tmp1[:], MV1[:, 0:8:2], Amu[:], op=A.mult)
    nc.vector.tensor_tensor(Bmu[:], P1[:, 1:16:4], tmp1[:], op=A.subtract)
    nc.vector.tensor_tensor(Alv[:], inv1[:], P1[:, 2:16:4], op=A.mult)
    nc.vector.tensor_tensor(tmp2[:], MV1[:, 0:8:2], Alv[:], op=A.mult)
    nc.vector.tensor_tensor(Blv[:], P1[:, 3:16:4], tmp2[:], op=A.subtract)

    # xn in bf16 (feeds the matmul): mu-head on GpSimd, lv-head on ACT
    XNmu, XNlv = [], []
    for k in range(4):
        t = sb.tile([128, N], BF16, tag=f"xnmu{k}", name=f"xnmu{k}")
        nc.gpsimd.tensor_scalar(
            t[:], Xt[k][:], Amu[:, k:k + 1], Bmu[:, k:k + 1],
            op0=A.mult, op1=A.add,
        )
        XNmu.append(t)
        t = sb.tile([128, N], BF16, tag=f"xnlv{k}", name=f"xnlv{k}")
        nc.scalar.activation(
            t[:], Xt[k][:], AF.Identity,
            bias=Blv[:, k:k + 1], scale=Alv[:, k:k + 1],
        )
        XNlv.append(t)
    XN = [XNmu, XNlv]

    # ---- mm1 + ReLU + BN2 + hn -----------------------------------------
    H = [
        sb.tile([128, N], F32, tag="hmu", name="hmu"),
        sb.tile([128, N], F32, tag="hlv", name="hlv"),
    ]
    MV2 = sb.tile([128, 4], F32, tag="mv2")
    for head in range(2):
        for half in range(2):
            pm = ps1.tile([128, 512], F32, tag="mm1", name=f"mm1_{head}{half}")
            for k in range(4):
                nc.tensor.matmul(
                    pm[:],
                    lhsT=W1b[k][:, head * HS:(head + 1) * HS],
                    rhs=XN[head][k][:, half * 512:(half + 1) * 512],
                    start=(k == 0), stop=(k == 3),
                )
            nc.scalar.activation(
                H[head][:, half * 512:(half + 1) * 512], pm[:], AF.Relu,
                bias=V2[:, head:head + 1], scale=1.0,
            )
        s6h = sb.tile([128, 12], F32, tag=f"s6h{head}", name=f"s6h{head}")
        for h in range(2):
            nc.vector.bn_stats(
                s6h[:, 6 * h:6 * h + 6], H[head][:, 512 * h:512 * (h + 1)]
            )
        nc.vector.bn_aggr(MV2[:, 2 * head:2 * head + 2], s6h[:])

    vp2 = sb.tile([128, 2], F32, tag="vp2")
    nc.vector.tensor_scalar_add(vp2[:], MV2[:, 1:4:2], EPS)
    rc2 = sb.tile([128, 2], F32, tag="rc2")
    nc.vector.reciprocal(rc2[:], vp2[:])
    inv2 = sb.tile([128, 2], F32, tag="inv2")
    nc.scalar.sqrt(inv2[:], rc2[:])

    A2 = sb.tile([128, 2], F32, tag="a2")
    B2 = sb.tile([128, 2], F32, tag="b2")
    tmp3 = sb.tile([128, 2], F32, tag="tmp3")
    # A2[:,h] = inv2[:,h]*g2h ; B2[:,h] = b2h - mean2h*A2[:,h]
    nc.vector.tensor_tensor(A2[:], inv2[:], V2[:, 2:6:2], op=A.mult)
    nc.vector.tensor_tensor(tmp3[:], MV2[:, 0:4:2], A2[:], op=A.mult)
    nc.vector.tensor_tensor(B2[:], V2[:, 3:6:2], tmp3[:], op=A.subtract)

    HN = [
        sb.tile([128, N], BF16, tag="hnmu", name="hnmu"),
        sb.tile([128, N], BF16, tag="hnlv", name="hnlv"),
    ]
    nc.gpsimd.tensor_scalar(
        HN[0][:], H[0][:], A2[:, 0:1], B2[:, 0:1], op0=A.mult, op1=A.add
    )
    nc.scalar.activation(
        HN[1][:], H[1][:], AF.Identity, bias=B2[:, 1:2], scale=A2[:, 1:2]
    )

    # ---- mm2: batch-major partials, both heads -------------------------
    # n = head*8 + j  ->  PO[n//4][:, (n%4)*128 : ...]
    PO = [ps2.tile([128, 512], F32, tag="po", name=f"po{t}") for t in range(4)]
    for n in range(16):
        head, j = divmod(n, 8)
        t, q = divmod(n, 4)
        nc.tensor.matmul(
            PO[t][:, q * 128:(q + 1) * 128],
            lhsT=HN[head][:, j * 128:(j + 1) * 128],
            rhs=W2b[:, head * YD:(head + 1) * YD],
            start=True, stop=True,
        )
    OUTS = sb.tile([128, 2048], F32, tag="outs")
    for t in range(4):
        if t % 2 == 0:
            nc.vector.tensor_copy(OUTS[:, t * 512:(t + 1) * 512], PO[t][:])
        else:
            nc.scalar.copy(OUTS[:, t * 512:(t + 1) * 512], PO[t][:])

    cc_in = dram.tile([N, 2 * YD], F32, tag="cc_in")
    cc_out = dram.tile([N, 2 * YD], F32, tag="cc_out")
    for n in range(16):
        head, j = divmod(n, 8)
        nc.sync.dma_start(
            cc_in[j * BS:(j + 1) * BS, head * YD:(head + 1) * YD],
            OUTS[:, n * 128:(n + 1) * 128],
        )
    nc.gpsimd.collective_compute(
        "AllToAll",
        A.bypass,
        replica_groups=[list(range(NCORES))],
        ins=[cc_in[:].opt()],
        outs=[cc_out[:].opt()],
    )

    # ---- y stats (overlaps everything above) ---------------------------
    PYS = psm.tile([1, 256], F32, tag="aux")
    for j in range(8):
        ysq = sb.tile([128, 256], F32, tag=f"ysq{j}", name=f"ysq{j}")
        nc.gpsimd.dma_start(ysq[:, 0:128], YN[j * 128:(j + 1) * 128, :])
        nc.scalar.square(ysq[:, 128:256], ysq[:, 0:128])
        nc.tensor.matmul(
            PYS[:], lhsT=ones_col[:], rhs=ysq[:], start=(j == 0), stop=(j == 7)
        )
    # Ey and VarY rows
    nc.scalar.mul(row512[:, 256:384], PYS[:, 0:128], 1.0 / N)
    ey2 = sb.tile([1, 128], F32, tag="ey2")
    nc.scalar.mul(ey2[:], PYS[:, 128:256], 1.0 / N)
    eysq = sb.tile([1, 128], F32, tag="eysq")
    nc.vector.tensor_tensor(eysq[:], row512[:, 256:384], row512[:, 256:384], op=A.mult)
    nc.vector.tensor_tensor(row512[:, 384:512], ey2[:], eysq[:], op=A.subtract)

    # broadcast [c2mu | c2lv | Ey | VarY] along 128 batch partitions
    BC = psm.tile([128, 512], F32, tag="bc")
    nc.tensor.matmul(BC[:], lhsT=ones_row[:], rhs=row512[:], start=True, stop=True)

    # ---- post-all-to-all: sum 8 partial slabs, then the loss tail ------
    RK = sb.tile([128, 8, 256], F32, tag="rk")
    nc.sync.dma_start(
        RK[:], cc_out[:].rearrange("(i p) c -> p i c", p=BS)
    )
    # pairwise tree sum over i: 4 + 2 + 1 adds, split DVE / GpSimd
    L1 = sb.tile([128, 4, 256], F32, tag="l1")
    for i in range(4):
        eng = nc.vector if i % 2 == 0 else nc.gpsimd
        eng.tensor_tensor(
            L1[:, i, :], RK[:, 2 * i, :], RK[:, 2 * i + 1, :], op=A.add
        )
    L2 = sb.tile([128, 2, 256], F32, tag="l2")
    nc.vector.tensor_tensor(L2[:, 0, :], L1[:, 0, :], L1[:, 1, :], op=A.add)
    nc.gpsimd.tensor_tensor(L2[:, 1, :], L1[:, 2, :], L1[:, 3, :], op=A.add)
    R = sb.tile([128, 256], F32, tag="rres")
    nc.vector.tensor_tensor(R[:], L2[:, 0, :], L2[:, 1, :], op=A.add)

    mu = sb.tile([BS, YD], F32, tag="mu")
    nc.vector.tensor_tensor(mu[:], R[:, 0:128], BC[:, 0:128], op=A.add)
    plv = sb.tile([BS, YD], F32, tag="plv")
    nc.vector.tensor_tensor(plv[:], R[:, 128:256], BC[:, 128:256], op=A.add)
    lvt = sb.tile([BS, YD], F32, tag="lvt")
    nc.scalar.activation(lvt[:], plv[:], AF.Tanh)
    E1 = sb.tile([BS, YD], F32, tag="e1")
    nc.scalar.activation(E1[:], lvt[:], AF.Exp, scale=-1.0)
    E2 = sb.tile([BS, YD], F32, tag="e2")
    nc.vector.tensor_tensor(E2[:], E1[:], E1[:], op=A.mult)

    dm = sb.tile([BS, YD], F32, tag="dm")
    nc.vector.tensor_tensor(dm[:], mu[:], BC[:, 256:384], op=A.subtract)
    q1 = sb.tile([BS, YD], F32, tag="q1")
    nc.vector.tensor_tensor(q1[:], dm[:], dm[:], op=A.mult)
    q2 = sb.tile([BS, YD], F32, tag="q2")
    nc.vector.tensor_tensor(q2[:], q1[:], BC[:, 384:512], op=A.add)

    scrA = sb.tile([BS, YD], F32, tag="scrA")
    nc.vector.tensor_tensor(scrA[:], E1[:], q2[:], op=A.mult)
    uacc = sb.tile([BS, 1], F32, tag="uacc")
    nc.vector.tensor_reduce(uacc[:], scrA[:], axis=mybir.AxisListType.X, op=A.add)
    dd = sb.tile([BS, YD], F32, tag="dd")
    nc.vector.tensor_tensor(dd[:], mu[:], YSt[:], op=A.subtract)
    wd = sb.tile([BS, YD], F32, tag="wd")
    nc.gpsimd.tensor_tensor(wd[:], E2[:], dd[:], op=A.mult)
    scrB = sb.tile([BS, YD], F32, tag="scrB")
    nc.vector.tensor_tensor(scrB[:], wd[:], dd[:], op=A.mult)
    vacc = sb.tile([BS, 1], F32, tag="vacc")
    nc.vector.tensor_reduce(vacc[:], scrB[:], axis=mybir.AxisListType.X, op=A.add)
    rl = sb.tile([BS, 1], F32, tag="rl")
    nc.vector.tensor_tensor(rl[:], uacc[:], vacc[:], op=A.subtract)

    PF = psm.tile([1, 1], F32, tag="aux")
    nc.tensor.matmul(PF[:], lhsT=rl[:], rhs=ones_col[:], start=True, stop=True)
    res = sb.tile([1, 1], F32, tag="res")
    nc.scalar.mul(res[:], PF[:], 0.5 / N)
    nc.sync.dma_start(out_ap[:, :], res[:])


_NC_CACHE = {}


def build(stage=99):
    if stage in _NC_CACHE:
        return _NC_CACHE[stage]
    nc = bacc.Bacc(
        "TRN2", target_bir_lowering=False, debug=False, num_devices=NCORES
    )
    io = {}

    def inp(name, shape):
        io[name] = nc.dram_tensor(name, list(shape), F32, kind="ExternalInput").ap()

    inp("xT", (XD, N))
    inp("yN", (N, YD))
    inp("ys", (BS, YD))
    inp("w1", (XD, 2 * HS))
    inp("w2", (HS, 2 * YD))
    inp("g1b1", (XD, 4))
    inp("vec2", (HS, 6))
    inp("c2row", (1, 2 * YD))
    out_ap = nc.dram_tensor("out", [1, 1], F32, kind="ExternalOutput").ap()

    with tile.TileContext(nc) as tc, ExitStack() as ctx:
        _program(ctx, tc, io, out_ap)
    nc.compile()
    _NC_CACHE[stage] = nc
    return nc


def make_in_maps(
    x_samples, y_samples,
    mu_g1, mu_b1, mu_W1, mu_c1, mu_g2, mu_b2, mu_W2, mu_c2,
    lv_g1, lv_b1, lv_W1, lv_c1, lv_g2, lv_b2, lv_W2, lv_c2,
):
    f = np.float32
    xT = np.ascontiguousarray(np.asarray(x_samples, f).T)          # [512, 1024]
    yN = np.ascontiguousarray(np.asarray(y_samples, f))            # [1024, 128]
    g1b1 = np.ascontiguousarray(
        np.stack([mu_g1, mu_b1, lv_g1, lv_b1], axis=1).astype(f)
    )                                                              # [512, 4]
    c2row = np.ascontiguousarray(
        np.concatenate([mu_c2, lv_c2])[None, :].astype(f)
    )                                                              # [1, 256]
    in_maps = []
    for c in range(NCORES):
        hs = slice(c * HS, (c + 1) * HS)
        bs = slice(c * BS, (c + 1) * BS)
        w1 = np.ascontiguousarray(
            np.concatenate([mu_W1[:, hs], lv_W1[:, hs]], axis=1).astype(f)
        )                                                          # [512, 256]
        w2 = np.ascontiguousarray(
            np.concatenate([mu_W2[hs, :], lv_W2[hs, :]], axis=1).astype(f)
        )                                                          # [128, 256]
        vec2 = np.ascontiguousarray(
            np.stack(
                [mu_c1[hs], lv_c1[hs], mu_g2[hs], mu_b2[hs], lv_g2[hs], lv_b2[hs]],
                axis=1,
            ).astype(f)
        )                                                          # [128, 6]
        ys = np.ascontiguousarray(yN[bs, :])                       # [128, 128]
        in_maps.append(
            dict(xT=xT, yN=yN, ys=ys, w1=w1, w2=w2, g1b1=g1b1, vec2=vec2, c2row=c2row)
        )
    return in_maps


def run_on_hw(in_maps, trace=False, stage=99, **kw):
    nc = build(stage)
    return run_bass_kernel_spmd(nc, in_maps, list(range(NCORES)), trace=trace, **kw)


def kernel(**inputs) -> np.ndarray:
    in_maps = make_in_maps(**inputs)
    res = run_on_hw(in_maps)
    total = np.float32(0.0)
    for r in res.results:
        total += np.float32(r["out"].reshape(-1)[0])
    return np.asarray(total, dtype=np.float32)


# revision 26
# speedup vs baseline: 1.1269x; 1.1106x over previous
"""CLUB loss kernel for Trainium2, 8 NeuronCores.

Math (reference semantics):
  mu     = head_mu(x)            # BN -> Linear(512,1024) -> ReLU -> BN -> Linear(1024,128)
  logvar = tanh(head_lv(x))
  positive[i,d] = -(mu-y)^2 * 0.5 * exp(-2 lv)
  pair_mse[i,d] = mean_j (y[j,d]-mu[i,d])^2
                = (mu[i,d]-Ey[d])^2 + VarY[d]          (exact algebraic identity)
  negative      = -pair_mse * 0.5 * exp(-lv)
  loss = mean_i( sum_d positive - sum_d negative )

Sharding: model-parallel over the hidden dim (1024 = 8 x 128).  Every core
reads full x (transposed) and computes BN1 stats redundantly (cheaper than a
stats collective), its 128-hidden slice of h/BN2/hn, and rank-1 partial
products for the second matmuls of both heads.  Partials for both heads,
laid out batch-major [1024, 256], go through one AllToAll; each core then
sums the 8 received partial slabs for its own batch shard and computes the
loss tail locally.  Per-core partial sums are summed on the host (unshard).

Matmuls run in bf16 (fp32 matmul is quarter-rate on PE); batch-norm
statistics and all loss arithmetic stay fp32.
"""

import numpy as np
from contextlib import ExitStack

import concourse.bass as bass
import concourse.bacc as bacc
import concourse.tile as tile
import concourse.mybir as mybir
from concourse.bass_utils import run_bass_kernel_spmd

N, XD, YD, HID = 1024, 512, 128, 1024
NCORES = 8
HS = HID // NCORES     # hidden slice per core
BS = N // NCORES       # batch rows per core after all-to-all
EPS = 1e-5
F32 = mybir.dt.float32
BF16 = mybir.dt.bfloat16


def _program(ctx, tc, io, out_ap):
    nc = tc.nc
    A = mybir.AluOpType
    AF = mybir.ActivationFunctionType
    XT, YN, YS, W1, W2, G1B1, V2D, C2R = (
        io[k] for k in ["xT", "yN", "ys", "w1", "w2", "g1b1", "vec2", "c2row"]
    )

    sb = ctx.enter_context(tc.tile_pool(name="sb", bufs=1))
    ps1 = ctx.enter_context(tc.tile_pool(name="ps1", bufs=2, space="PSUM"))
    ps2 = ctx.enter_context(tc.tile_pool(name="ps2", bufs=4, space="PSUM"))
    psm = ctx.enter_context(tc.tile_pool(name="psm", bufs=1, space="PSUM"))
    dram = ctx.enter_context(tc.tile_pool(name="dram", bufs=1, space="DRAM"))

    # ---- x load first (critical path), split across both HWDGE rings ---
    Xt = []
    for k in range(4):
        t = sb.tile([128, N], F32, tag=f"x{k}", name=f"x{k}")
        eng = nc.sync if k < 2 else nc.scalar
        eng.dma_start(t[:], XT[128 * k:128 * (k + 1), :])
        Xt.append(t)

    # ---- params ---------------------------------------------------------
    ones_col = sb.tile([128, 1], F32, tag="ones_col")
    nc.vector.memset(ones_col[:], 1.0)
    ones_row = sb.tile([1, 128], F32, tag="ones_row")
    nc.vector.memset(ones_row[:], 1.0)

    P1 = sb.tile([128, 16], F32, tag="p1")       # (g1mu,b1mu,g1lv,b1lv) x 4 chunks
    for k in range(4):
        nc.sync.dma_start(P1[:, 4 * k:4 * k + 4], G1B1[128 * k:128 * (k + 1), :])
    V2 = sb.tile([128, 6], F32, tag="v2")        # c1mu,c1lv,g2mu,b2mu,g2lv,b2lv
    nc.sync.dma_start(V2[:], V2D[:, :])
    row512 = sb.tile([1, 512], F32, tag="row512")  # [c2mu | c2lv | Ey | VarY]
    nc.sync.dma_start(row512[:, 0:256], C2R[:, :])

    W1b = []
    for k in range(4):
        t32 = sb.tile([128, 2 * HS], F32, tag="w1f32", name=f"w1f{k}")
        nc.scalar.dma_start(t32[:], W1[128 * k:128 * (k + 1), :])
        tb = sb.tile([128, 2 * HS], BF16, tag=f"w1b{k}", name=f"w1b{k}")
        eng = nc.vector if k % 2 == 0 else nc.gpsimd
        eng.tensor_copy(tb[:], t32[:])
        W1b.append(tb)
    W2f = sb.tile([128, 2 * YD], F32, tag="w2f")
    nc.scalar.dma_start(W2f[:], W2[:, :])
    W2b = sb.tile([128, 2 * YD], BF16, tag="w2b")
    nc.gpsimd.tensor_copy(W2b[:], W2f[:])
    YSt = sb.tile([BS, YD], F32, tag="ys")
    nc.sync.dma_start(YSt[:], YS[:, :])

    # ---- BN1 ------------------------------------------------------------
    MV1 = sb.tile([128, 8], F32, tag="mv1")      # (mean, var) x 4 chunks
    for k in range(4):
        s6 = sb.tile([128, 12], F32, tag=f"s6_{k}", name=f"s6_{k}")
        for h in range(2):
            nc.vector.bn_stats(
                s6[:, 6 * h:6 * h + 6], Xt[k][:, 512 * h:512 * (h + 1)]
            )
        nc.vector.bn_aggr(MV1[:, 2 * k:2 * k + 2], s6[:])

    # invstd for all 4 chunks at once: [128, 4]
    vp1 = sb.tile([128, 4], F32, tag="vp1")
    nc.vector.tensor_scalar_add(vp1[:], MV1[:, 1:8:2], EPS)
    rc1 = sb.tile([128, 4], F32, tag="rc1")
    nc.vector.reciprocal(rc1[:], vp1[:])
    inv1 = sb.tile([128, 4], F32, tag="inv1")
    nc.scalar.sqrt(inv1[:], rc1[:])

    # per-head affine  xn = x*Ah + Bh ;  Ah = invstd*g1h, Bh = b1h - mean*Ah
    Amu = sb.tile([128, 4], F32, tag="amu")
    Bmu = sb.tile([128, 4], F32, tag="bmu")
    Alv = sb.tile([128, 4], F32, tag="alv")
    Blv = sb.tile([128, 4], F32, tag="blv")
    tmp1 = sb.tile([128, 4], F32, tag="tmp1")
    tmp2 = sb.tile([128, 4], F32, tag="tmp2")
    nc.vector.tensor_tensor(Amu[:], inv1[:], P1[:, 0:16:4], op=A.mult)
    nc.vector.tensor_tensor(tmp1[:], MV1[:, 0:8:2], Amu[:], op=A.mult)
    nc.vector.tensor_tensor(Bmu[:], P1[:, 1:16:4], tmp1[:], op=A.subtract)
    nc.vector.tensor_tensor(Alv[:], inv1[:], P1[:, 2:16:4], op=A.mult)
    nc.vector.tensor_tensor(tmp2[:], MV1[:, 0:8:2], Alv[:], op=A.mult)
    nc.vector.tensor_tensor(Blv[:], P1[:, 3:16:4], tmp2[:], op=A.subtract)

    # xn in bf16 (feeds the matmul): mu-head on GpSimd, lv-head on ACT
    XNmu, XNlv = [], []
    for k in range(4):
        t = sb.tile([128, N], BF16, tag=f"xnmu{k}", name=f"xnmu{k}")
        nc.gpsimd.tensor_scalar(
            t[:], Xt[k][:], Amu[:, k:k + 1], Bmu[:, k:k + 1],
            op0=A.mult, op1=A.add,
        )
        XNmu.append(t)
        t = sb.tile([128, N], BF16, tag=f"xnlv{k}", name=f"xnlv{k}")
        nc.scalar.activation(
            t[:], Xt[k][:], AF.Identity,
            bias=Blv[:, k:k + 1], scale=Alv[:, k:k + 1],
        )
        XNlv.append(t)
    XN = [XNmu, XNlv]

    # ---- mm1 + ReLU + BN2 + hn -----------------------------------------
    H = [
        sb.tile([128, N], F32, tag="hmu", name="hmu"),
        sb.tile([128, N], F32, tag="hlv", name="hlv"),
    ]
    MV2 = sb.tile([128, 4], F32, tag="mv2")
    for head in range(2):
        for half in range(2):
            pm = ps1.tile([128, 512], F32, tag="mm1", name=f"mm1_{head}{half}")
            for k in range(4):
                nc.tensor.matmul(
                    pm[:],
                    lhsT=W1b[k][:, head * HS:(head + 1) * HS],
                    rhs=XN[head][k][:, half * 512:(half + 1) * 512],
                    start=(k == 0), stop=(k == 3),
                )
            nc.scalar.activation(
                H[head][:, half * 512:(half + 1) * 512], pm[:], AF.Relu,
                bias=V2[:, head:head + 1], scale=1.0,
            )
        s6h = sb.tile([128, 12], F32, tag=f"s6h{head}", name=f"s6h{head}")
        for h in range(2):
            nc.vector.bn_stats(
                s6h[:, 6 * h:6 * h + 6], H[head][:, 512 * h:512 * (h + 1)]
            )
        nc.vector.bn_aggr(MV2[:, 2 * head:2 * head + 2], s6h[:])

    vp2 = sb.tile([128, 2], F32, tag="vp2")
    nc.vector.tensor_scalar_add(vp2[:], MV2[:, 1:4:2], EPS)
    rc2 = sb.tile([128, 2], F32, tag="rc2")
    nc.vector.reciprocal(rc2[:], vp2[:])
    inv2 = sb.tile([128, 2], F32, tag="inv2")
    nc.scalar.sqrt(inv2[:], rc2[:])

    A2 = sb.tile([128, 2], F32, tag="a2")
    B2 = sb.tile([128, 2], F32, tag="b2")
    tmp3 = sb.tile([128, 2], F32, tag="tmp3")
    # A2[:,h] = inv2[:,h]*g2h ; B2[:,h] = b2h - mean2h*A2[:,h]
    nc.vector.tensor_tensor(A2[:], inv2[:], V2[:, 2:6:2], op=A.mult)
    nc.vector.tensor_tensor(tmp3[:], MV2[:, 0:4:2], A2[:], op=A.mult)
    nc.vector.tensor_tensor(B2[:], V2[:, 3:6:2], tmp3[:], op=A.subtract)

    HN = [
        sb.tile([128, N], BF16, tag="hnmu", name="hnmu"),
        sb.tile([128, N], BF16, tag="hnlv", name="hnlv"),
    ]
    nc.gpsimd.tensor_scalar(
        HN[0][:], H[0][:], A2[:, 0:1], B2[:, 0:1], op0=A.mult, op1=A.add
    )
    nc.vector.tensor_scalar(
        HN[1][:], H[1][:], A2[:, 1:2], B2[:, 1:2], op0=A.mult, op1=A.add
    )

    # ---- mm2: batch-major partials, both heads -------------------------
    # n = head*8 + j  ->  PO[n//4][:, (n%4)*128 : ...]
    PO = [ps2.tile([128, 512], F32, tag="po", name=f"po{t}") for t in range(4)]
    for n in range(16):
        head, j = divmod(n, 8)
        t, q = divmod(n, 4)
        nc.tensor.matmul(
            PO[t][:, q * 128:(q + 1) * 128],
            lhsT=HN[head][:, j * 128:(j + 1) * 128],
            rhs=W2b[:, head * YD:(head + 1) * YD],
            start=True, stop=True,
        )
    OUTS = sb.tile([128, 2048], BF16, tag="outs")
    for t in range(4):
        if t % 2 == 0:
            nc.vector.tensor_copy(OUTS[:, t * 512:(t + 1) * 512], PO[t][:])
        else:
            nc.scalar.copy(OUTS[:, t * 512:(t + 1) * 512], PO[t][:])

    cc_in = dram.tile([N, 2 * YD], BF16, tag="cc_in")
    cc_out = dram.tile([N, 2 * YD], BF16, tag="cc_out")
    for n in range(16):
        head, j = divmod(n, 8)
        nc.sync.dma_start(
            cc_in[j * BS:(j + 1) * BS, head * YD:(head + 1) * YD],
            OUTS[:, n * 128:(n + 1) * 128],
        )
    nc.gpsimd.collective_compute(
        "AllToAll",
        A.bypass,
        replica_groups=[list(range(NCORES))],
        ins=[cc_in[:].opt()],
        outs=[cc_out[:].opt()],
    )

    # ---- y stats (overlaps everything above) ---------------------------
    PYS = psm.tile([1, 256], F32, tag="aux")
    for j in range(8):
        ysq = sb.tile([128, 256], F32, tag=f"ysq{j}", name=f"ysq{j}")
        nc.gpsimd.dma_start(ysq[:, 0:128], YN[j * 128:(j + 1) * 128, :])
        nc.gpsimd.tensor_tensor(
            ysq[:, 128:256], ysq[:, 0:128], ysq[:, 0:128], op=A.mult
        )
        nc.tensor.matmul(
            PYS[:], lhsT=ones_col[:], rhs=ysq[:], start=(j == 0), stop=(j == 7)
        )
    # Ey and VarY rows
    nc.scalar.mul(row512[:, 256:384], PYS[:, 0:128], 1.0 / N)
    ey2 = sb.tile([1, 128], F32, tag="ey2")
    nc.scalar.mul(ey2[:], PYS[:, 128:256], 1.0 / N)
    eysq = sb.tile([1, 128], F32, tag="eysq")
    nc.vector.tensor_tensor(eysq[:], row512[:, 256:384], row512[:, 256:384], op=A.mult)
    nc.vector.tensor_tensor(row512[:, 384:512], ey2[:], eysq[:], op=A.subtract)

    # broadcast [c2mu | c2lv | Ey | VarY] along 128 batch partitions
    BC = psm.tile([128, 512], F32, tag="bc")
    nc.tensor.matmul(BC[:], lhsT=ones_row[:], rhs=row512[:], start=True, stop=True)

    # ---- post-all-to-all: sum 8 partial slabs, then the loss tail ------
    RK = sb.tile([128, 8, 256], BF16, tag="rk")
    nc.sync.dma_start(
        RK[:], cc_out[:].rearrange("(i p) c -> p i c", p=BS)
    )
    # pairwise tree sum over i: 4 + 2 + 1 adds, split DVE / GpSimd
    L1 = sb.tile([128, 4, 256], F32, tag="l1")
    for i in range(4):
        eng = nc.vector if i % 2 == 0 else nc.gpsimd
        eng.tensor_tensor(
            L1[:, i, :], RK[:, 2 * i, :], RK[:, 2 * i + 1, :], op=A.add
        )
    L2 = sb.tile([128, 2, 256], F32, tag="l2")
    nc.vector.tensor_tensor(L2[:, 0, :], L1[:, 0, :], L1[:, 1, :], op=A.add)
    nc.gpsimd.tensor_tensor(L2[:, 1, :], L1[:, 2, :], L1[:, 3, :], op=A.add)
    R = sb.tile([128, 256], F32, tag="rres")
    nc.vector.tensor_tensor(R[:], L2[:, 0, :], L2[:, 1, :], op=A.add)

    mu = sb.tile([BS, YD], F32, tag="mu")
    nc.vector.tensor_tensor(mu[:], R[:, 0:128], BC[:, 0:128], op=A.add)
    plv = sb.tile([BS, YD], F32, tag="plv")
    nc.vector.tensor_tensor(plv[:], R[:, 128:256], BC[:, 128:256], op=A.add)
    lvt = sb.tile([BS, YD], F32, tag="lvt")
    nc.scalar.activation(lvt[:], plv[:], AF.Tanh)
    E1 = sb.tile([BS, YD], F32, tag="e1")
    nc.scalar.activation(E1[:], lvt[:], AF.Exp, scale=-1.0)
    E2 = sb.tile([BS, YD], F32, tag="e2")
    nc.vector.tensor_tensor(E2[:], E1[:], E1[:], op=A.mult)

    dm = sb.tile([BS, YD], F32, tag="dm")
    nc.vector.tensor_tensor(dm[:], mu[:], BC[:, 256:384], op=A.subtract)
    q1 = sb.tile([BS, YD], F32, tag="q1")
    nc.vector.tensor_tensor(q1[:], dm[:], dm[:], op=A.mult)
    q2 = sb.tile([BS, YD], F32, tag="q2")
    nc.vector.tensor_tensor(q2[:], q1[:], BC[:, 384:512], op=A.add)

    scrA = sb.tile([BS, YD], F32, tag="scrA")
    nc.vector.tensor_tensor(scrA[:], E1[:], q2[:], op=A.mult)
    uacc = sb.tile([BS, 1], F32, tag="uacc")
    nc.vector.tensor_reduce(uacc[:], scrA[:], axis=mybir.AxisListType.X, op=A.add)
    dd = sb.tile([BS, YD], F32, tag="dd")
    nc.vector.tensor_tensor(dd[:], mu[:], YSt[:], op=A.subtract)
    wd = sb.tile([BS, YD], F32, tag="wd")
    nc.gpsimd.tensor_tensor(wd[:], E2[:], dd[:], op=A.mult)
    scrB = sb.tile([BS, YD], F32, tag="scrB")
    nc.vector.tensor_tensor(scrB[:], wd[:], dd[:], op=A.mult)
    vacc = sb.tile([BS, 1], F32, tag="vacc")
    nc.vector.tensor_reduce(vacc[:], scrB[:], axis=mybir.AxisListType.X, op=A.add)
    rl = sb.tile([BS, 1], F32, tag="rl")
    nc.vector.tensor_tensor(rl[:], uacc[:], vacc[:], op=A.subtract)

    PF = psm.tile([1, 1], F32, tag="aux")
    nc.tensor.matmul(PF[:], lhsT=rl[:], rhs=ones_col[:], start=True, stop=True)
    res = sb.tile([1, 1], F32, tag="res")
    nc.scalar.mul(res[:], PF[:], 0.5 / N)
    nc.sync.dma_start(out_ap[:, :], res[:])


_NC_CACHE = {}


def build(stage=99):
    if stage in _NC_CACHE:
        return _NC_CACHE[stage]
    nc = bacc.Bacc(
        "TRN2", target_bir_lowering=False, debug=False, num_devices=NCORES
    )
    io = {}

    def inp(name, shape):
        io[name] = nc.dram_tensor(name, list(shape), F32, kind="ExternalInput").ap()

    inp("xT", (XD, N))
    inp("yN", (N, YD))
    inp("ys", (BS, YD))
    inp("w1", (XD, 2 * HS))
    inp("w2", (HS, 2 * YD))
    inp("g1b1", (XD, 4))
    inp("vec2", (HS, 6))
    inp("c2row", (1, 2 * YD))
    out_ap = nc.dram_tensor("out", [1, 1], F32, kind="ExternalOutput").ap()

    with tile.TileContext(nc) as tc, ExitStack() as ctx:
        _program(ctx, tc, io, out_ap)
    nc.compile()
    _NC_CACHE[stage] = nc
    return nc


def make_in_maps(
    x_samples, y_samples,
    mu_g1, mu_b1, mu_W1, mu_c1, mu_g2, mu_b2, mu_W2, mu_c2,
    lv_g1, lv_b1, lv_W1, lv_c1, lv_g2, lv_b2, lv_W2, lv_c2,
):
    f = np.float32
    xT = np.ascontiguousarray(np.asarray(x_samples, f).T)          # [512, 1024]
    yN = np.ascontiguousarray(np.asarray(y_samples, f))            # [1024, 128]
    g1b1 = np.ascontiguousarray(
        np.stack([mu_g1, mu_b1, lv_g1, lv_b1], axis=1).astype(f)
    )                                                              # [512, 4]
    c2row = np.ascontiguousarray(
        np.concatenate([mu_c2, lv_c2])[None, :].astype(f)
    )                                                              # [1, 256]
    in_maps = []
    for c in range(NCORES):
        hs = slice(c * HS, (c + 1) * HS)
        bs = slice(c * BS, (c + 1) * BS)
        w1 = np.ascontiguousarray(
            np.concatenate([mu_W1[:, hs], lv_W1[:, hs]], axis=1).astype(f)
        )                                                          # [512, 256]
        w2 = np.ascontiguousarray(
            np.concatenate([mu_W2[hs, :], lv_W2[hs, :]], axis=1).astype(f)
        )                                                          # [128, 256]
        vec2 = np.ascontiguousarray(
            np.stack(
                [mu_c1[hs], lv_c1[hs], mu_g2[hs], mu_b2[hs], lv_g2[hs], lv_b2[hs]],
                axis=1,
            ).astype(f)
        )                                                          # [128, 6]
        ys = np.ascontiguousarray(yN[bs, :])                       # [128, 128]
        in_maps.append(
            dict(xT=xT, yN=yN, ys=ys, w1=w1, w2=w2, g1b1=g1b1, vec2=vec2, c2row=c2row)
        )
    return in_maps


def run_on_hw(in_maps, trace=False, stage=99, **kw):
    nc = build(stage)
    return run_bass_kernel_spmd(nc, in_maps, list(range(NCORES)), trace=trace, **kw)


def kernel(**inputs) -> np.ndarray:
    in_maps = make_in_maps(**inputs)
    res = run_on_hw(in_maps)
    total = np.float32(0.0)
    for r in res.results:
        total += np.float32(r["out"].reshape(-1)[0])
    return np.asarray(total, dtype=np.float32)
